# revision 27
# baseline (speedup 1.0000x reference)
"""Baichuan attention (B=2, S=2048, H=4096, 32 heads x 128) on 8 TRN2 NeuronCores.

Sharding: tensor-parallel over heads (4 heads per core), as in the original
model: W_pack column-sharded (per-head rows), o_proj row-sharded with the
partial-sum reduction done on the host during unshard ("all-reduce" of the
row-parallel output).

Fused single-region pipeline per core (all matmuls bf16), with attention
chunks interleaved into the dense QKV / o_proj matmul streams at emission
time so the PE queue always has independent work behind exp-latency stalls:

  qkv(b0) | attn(b0) x qkv(b1) | attn(b1) x oproj(b0) | oproj(b1)

  1. QKV projection from x^T: per (b, s-block): q/k passes as (qk, dt, sh)
     sub-passes - one [128,512] psum bank per sub-pass, stationary =
     [128,128] w-slab slice, LDW fully hidden at the 216ns N=512 cadence.
     RoPE on the fly (partition-swap via SBUF-SBUF DMA on the gpsimd
     queue).  qT/kT/v written bf16 to DRAM scratch.
  2. Attention per (b, h, qc), kt software-pipelined: scores matmul for
     kt+1 issued ahead of ctx/den matmuls for kt.  Denominator via all-ones
     stationary matmul (broadcast across partitions),
     reciprocal_approx_fast + normalize on VectorE.
  3. o_proj: ctx-stationary, drains alternate Scalar/Vector, bf16
     out-writes on the gpsimd DMA queue.
Host: shards/transposes inputs, sums the 8 row-parallel bf16 partials.
"""
import os
import sys

for _p in ("/opt/trn_rl_repo", "/root/.axon_site/_ro/trn_rl_repo"):
    if os.path.isdir(_p) and _p not in sys.path:
        sys.path.insert(0, _p)

from contextlib import ExitStack

import ml_dtypes
import numpy as np

import concourse.bass as bass
import concourse.tile as tile
from concourse import bacc, bass_isa, mybir
from concourse.bass_utils import run_bass_kernel_spmd

F32 = mybir.dt.float32
F32R = mybir.dt.float32r
BF16 = mybir.dt.bfloat16

B, S, H = 2, 2048, 4096
NH, HD = 32, 128
NCORES = 8
HPC = NH // NCORES          # heads per core = 4
DPC = HPC * HD              # dims per core = 512
ROPE_BASE = 10000.0
NEG = -1.0e30

SBLK = 1024                 # projection s-block
NSB = S // SBLK             # 2 s-blocks per batch
QC = 512                    # attention q-chunk
NQC = S // QC               # 4 q-chunks
NHT = H // 128              # 32 h-tiles (contraction tiles)
NKT = S // 128              # 16 k-tiles per sequence


def _build():
    nc = bacc.Bacc("TRN2", target_bir_lowering=False, debug=False,
                   num_devices=NCORES)

    xT = nc.dram_tensor("xT", [B, H, S], BF16, kind="ExternalInput").ap()
    # w slabs: [qkv(3) * dt(4)] slabs, each [128 (h-part), NHT, 128 (cols)]
    wqk = nc.dram_tensor("wqk", [12, 128, NHT, 128], BF16,
                         kind="ExternalInput").ap()
    woT = nc.dram_tensor("woT", [DPC, H], BF16, kind="ExternalInput").ap()
    cosT = nc.dram_tensor("cosT", [HD, S], BF16, kind="ExternalInput").ap()
    sinTm = nc.dram_tensor("sinTm", [HD, S], BF16, kind="ExternalInput").ap()
    masks = nc.dram_tensor("masks", [4, 128, QC], BF16,
                           kind="ExternalInput").ap()

    qkT_s = nc.dram_tensor("qkT_s", [B, 2 * DPC, S], BF16).ap()
    v_sT = nc.dram_tensor("v_sT", [B, DPC, S], BF16).ap()

    out = nc.dram_tensor("out", [B, S, H], BF16, kind="ExternalOutput").ap()

    with tile.TileContext(nc) as tc, ExitStack() as ctx:
        singles = ctx.enter_context(tc.tile_pool(name="singles", bufs=1))
        xpool = ctx.enter_context(tc.tile_pool(name="xslab", bufs=NHT))
        wpool = ctx.enter_context(tc.tile_pool(name="wslab", bufs=2))
        rpool = ctx.enter_context(tc.tile_pool(name="rope", bufs=2))
        opool = ctx.enter_context(tc.tile_pool(name="pj_out", bufs=3))
        qkvpool = ctx.enter_context(tc.tile_pool(name="at_qkv", bufs=1))
        ctxpool = ctx.enter_context(tc.tile_pool(name="at_ctx", bufs=2))
        prpool = ctx.enter_context(tc.tile_pool(name="at_pr", bufs=3))
        smpool = ctx.enter_context(tc.tile_pool(name="at_sm", bufs=2))
        wopool = ctx.enter_context(tc.tile_pool(name="at_wo", bufs=2))
        oopool = ctx.enter_context(tc.tile_pool(name="at_oo", bufs=4))
        # PSUM: 2 (qkv/v/oproj shared ring) + 2 scores + 2 ctx + 2 den = 8
        pp = ctx.enter_context(tc.tile_pool(name="ps_mm", bufs=2,
                                            space="PSUM"))
        ps_s = ctx.enter_context(tc.tile_pool(name="ps_s", bufs=3,
                                              space="PSUM"))
        ps_c = ctx.enter_context(tc.tile_pool(name="ps_c", bufs=2,
                                              space="PSUM"))
        ps_d = ctx.enter_context(tc.tile_pool(name="ps_d", bufs=1,
                                              space="PSUM"))

        cos_sb = singles.tile([HD, S], BF16)
        sin_sb = singles.tile([HD, S], BF16)
        nc.sync.dma_start(out=cos_sb[:], in_=cosT[:])
        nc.sync.dma_start(out=sin_sb[:], in_=sinTm[:])
        mask_sb = singles.tile([128, 4, QC], BF16)
        for dj in range(4):
            nc.sync.dma_start(out=mask_sb[:, dj, :], in_=masks[dj])
        ones_sb = singles.tile([128, 128], BF16)
        nc.vector.memset(ones_sb[:], 1.0)

        def drain_qk(ps, b, row0, s0):
            """rope(psum tile [128, 512]) -> qkT_s[b, row0:+128, s0:+512]"""
            sl = ps[:]  # [128, 512]
            cslice = cos_sb[:, s0:s0 + 512]
            mslice = sin_sb[:, s0:s0 + 512]
            t1 = rpool.tile([128, 512], F32, tag="t1")
            nc.vector.tensor_mul(t1[:], sl, cslice)
            qsb = rpool.tile([128, 512], BF16, tag="qsb")
            nc.scalar.copy(qsb[:], sl)
            qsw = rpool.tile([128, 512], BF16, tag="qsw")
            nc.gpsimd.dma_start(out=qsw[0:64, :], in_=qsb[64:128, :])
            nc.gpsimd.dma_start(out=qsw[64:128, :], in_=qsb[0:64, :])
            nc.vector.tensor_mul(qsw[:], qsw[:], mslice)
            qo = opool.tile([128, 512], BF16, tag="qo")
            nc.vector.tensor_add(qo[:], t1[:], qsw[:])
            nc.gpsimd.dma_start(out=qkT_s[b, row0:row0 + 128, s0:s0 + 512],
                                in_=qo[:])

        def qkv_units(b, sb):
            """Emission units (closures) for one (b, s-block)."""
            s0 = sb * SBLK
            xsl = []

            def load_x():
                for h in range(NHT):
                    xs = xpool.tile([128, SBLK], BF16, tag="xs", name="xs")
                    nc.sync.dma_start(
                        out=xs[:],
                        in_=xT[b, h * 128:(h + 1) * 128, s0:s0 + SBLK])
                    xsl.append(xs)

            units = [load_x]

            # q/k/v passes: one psum bank per (pass, dt, sh) sub-pass
            for qkv in range(3):
                for dt in range(HPC):
                    def mk(qkv=qkv, dt=dt):
                        def run():
                            w = wpool.tile([128, NHT, 128], BF16, tag="w")
                            nc.sync.dma_start(out=w[:],
                                              in_=wqk[qkv * HPC + dt])
                            for sh in range(2):
                                ps = pp.tile([128, 512], F32, tag="mm",
                                             name="pp")
                                for h in range(NHT):
                                    nc.tensor.matmul(
                                        ps[:],
                                        w[:, h, :],
                                        xsl[h][:, sh * 512:(sh + 1) * 512],
                                        start=(h == 0), stop=(h == NHT - 1))
                                if qkv < 2:
                                    drain_qk(ps, b, qkv * DPC + dt * 128,
                                             s0 + sh * 512)
                                else:
                                    vo = opool.tile([128, 512], BF16,
                                                    tag="vo")
                                    nc.scalar.copy(vo[:], ps[:])
                                    nc.gpsimd.dma_start(
                                        out=v_sT[b,
                                                 dt * 128:(dt + 1) * 128,
                                                 s0 + sh * 512:
                                                 s0 + (sh + 1) * 512],
                                        in_=vo[:])
                        return run
                    units.append(mk())
            return units

        def attn_units(b, qT_all, kT_all, v_sb, ctx_sb, den_pe=False):
            """Emission units: one per (qc, h) chunk, qc-major order."""
            units = []
            for qc in range(NQC):
                for h in range(HPC):
                    def mk(h=h, qc=qc):
                        def run():
                            q0 = qc * QC
                            nkt = 4 * qc + 4
                            pc = ps_c.tile([128, QC], F32, tag="pc",
                                           name="pc")
                            den = smpool.tile([128, QC], F32, tag="den")
                            if den_pe:
                                pden = ps_d.tile([128, QC], F32, tag="pd",
                                                 name="pd")

                            def w0_of(kt):
                                # first unmasked q column of this k-tile
                                return max(0, (kt - 4 * qc) * 128)

                            def acc_prev(prev):
                                # ctx matmul + den accumulation for slot kt
                                pkt, ppr = prev
                                w0 = w0_of(pkt)
                                nc.tensor.matmul(
                                    pc[:, w0:QC],
                                    v_sb[:, pkt, h * HD:(h + 1) * HD],
                                    ppr[:, w0:QC], start=(pkt == 0),
                                    stop=(pkt == nkt - 1))
                                if den_pe:
                                    nc.tensor.matmul(
                                        pden[:, w0:QC], ones_sb[:],
                                        ppr[:, w0:QC], start=(pkt == 0),
                                        stop=(pkt == nkt - 1))
                                elif pkt == 0:
                                    nc.vector.tensor_copy(den[:], ppr[:])
                                else:
                                    nc.vector.tensor_add(
                                        den[:, w0:QC], den[:, w0:QC],
                                        ppr[:, w0:QC])

                            prev = None
                            for kt in range(nkt):
                                w0 = w0_of(kt)
                                pss = ps_s.tile([128, QC], F32, tag="pss",
                                                name="pss")
                                nc.tensor.matmul(
                                    pss[:, w0:QC],
                                    kT_all[:, h, kt * 128:(kt + 1) * 128],
                                    qT_all[:, h, q0 + w0:q0 + QC],
                                    start=True, stop=True)
                                if kt >= 4 * qc:
                                    nc.vector.tensor_add(
                                        pss[:, w0:QC], pss[:, w0:QC],
                                        mask_sb[:, kt - 4 * qc, w0:QC])
                                pr = prpool.tile([128, QC], BF16, tag="pr",
                                                 name="pr")
                                nc.scalar.activation(
                                    out=pr[:, w0:QC], in_=pss[:, w0:QC],
                                    func=mybir.ActivationFunctionType.Exp)
                                if prev is not None:
                                    acc_prev(prev)
                                prev = (kt, pr)
                            acc_prev(prev)
                            if not den_pe:
                                den16 = smpool.tile([128, QC], BF16,
                                                    tag="den16")
                                nc.scalar.copy(den16[:], den[:])
                                pden = ps_d.tile([128, QC], F32, tag="pd",
                                                 name="pd")
                                nc.tensor.matmul(pden[:], ones_sb[:],
                                                 den16[:],
                                                 start=True, stop=True)
                            nc.vector.reciprocal_approx_fast(out=den[:],
                                                             in_=pden[:])
                            nc.vector.tensor_mul(
                                ctx_sb[:, h, q0:q0 + QC], pc[:], den[:])
                        return run
                    units.append(mk())
            return units

        def oproj_units(b, ctx_sb):
            units = []
            for oc in range(H // 512):
                def mk(oc=oc):
                    def run():
                        wos = wopool.tile([128, HPC, 512], BF16, tag="wos")
                        nc.sync.dma_start(
                            out=wos[:],
                            in_=woT[:, oc * 512:(oc + 1) * 512].rearrange(
                                "(h p) o -> p h o", p=128))
                        for st in range(S // 128):
                            po = pp.tile([128, 512], F32, tag="mm",
                                         name="po")
                            for h in range(HPC):
                                nc.tensor.matmul(
                                    po[:],
                                    ctx_sb[:, h, st * 128:(st + 1) * 128],
                                    wos[:, h, :],
                                    start=(h == 0), stop=(h == HPC - 1))
                            ot = oopool.tile([128, 512], BF16, tag="ot")
                            if st % 2 == 0:
                                nc.scalar.copy(ot[:], po[:])
                            else:
                                nc.vector.tensor_copy(ot[:], po[:])
                            nc.gpsimd.dma_start(
                                out=out[b, st * 128:(st + 1) * 128,
                                        oc * 512:(oc + 1) * 512],
                                in_=ot[:])
                    return run
                units.append(mk())
            return units

        def alloc_attn_inputs():
            qT_all = qkvpool.tile([128, HPC, S], BF16, tag="qT")
            kT_all = qkvpool.tile([128, HPC, S], BF16, tag="kT")
            v_sb = qkvpool.tile([128, NKT, DPC], BF16, tag="v")
            return qT_all, kT_all, v_sb

        def load_attn_inputs(b, tiles, qcs):
            # per-qc-slice loads, emitted only after the producing s-block
            qT_all, kT_all, v_sb = tiles
            for qc in qcs:
                sl = slice(qc * QC, (qc + 1) * QC)
                nc.sync.dma_start(
                    out=qT_all[:, :, sl],
                    in_=qkT_s[b, 0:DPC, sl].rearrange(
                        "(h p) s -> p h s", p=128))
                nc.sync.dma_start(
                    out=kT_all[:, :, sl],
                    in_=qkT_s[b, DPC:2 * DPC, sl].rearrange(
                        "(h p) s -> p h s", p=128))
                for kt in range(4 * qc, 4 * qc + 4):
                    nc.sync.dma_start_transpose(
                        out=v_sb[:, kt, :],
                        in_=v_sT[b][:, kt * 128:(kt + 1) * 128])

        def interleave(primary, secondary):
            """Emit primary units with secondary units spread between them."""
            if not secondary:
                for u in primary:
                    u()
                return
            ratio = len(primary) / len(secondary)
            si = 0
            for i, u in enumerate(primary):
                u()
                while si < len(secondary) and si + 1 <= (i + 1) / ratio:
                    secondary[si]()
                    si += 1
            while si < len(secondary):
                secondary[si]()
                si += 1

        # ---- fused schedule ----
        # qkv(0,0) | qkv(0,1) x attn0[qc01] | qkv(1) x attn0[qc23] |
        # oproj(0) x attn1 | oproj(1)
        for u in qkv_units(0, 0):
            u()
        at0 = alloc_attn_inputs()
        ctx0 = ctxpool.tile([128, HPC, S], BF16, tag="ctx")
        a0 = attn_units(0, *at0, ctx0)       # (qc, h) order, 4 per qc
        u01 = qkv_units(0, 1)
        u01[0]()                             # x loads for block (0,1)
        load_attn_inputs(0, at0, (0, 1))
        interleave(u01[1:], a0[:8])
        qkv1 = [u for sb in range(NSB) for u in qkv_units(1, sb)]
        qkv1[0]()                            # x loads for block (1,0)
        load_attn_inputs(0, at0, (2, 3))
        interleave(qkv1[1:], a0[8:])
        at1 = alloc_attn_inputs()
        ctx1 = ctxpool.tile([128, HPC, S], BF16, tag="ctx")
        op0 = oproj_units(0, ctx0)
        op0[0]()
        load_attn_inputs(1, at1, (0, 1, 2, 3))
        interleave(op0[1:], attn_units(1, *at1, ctx1, den_pe=True))
        for u in oproj_units(1, ctx1):
            u()

    nc.compile()
    return nc


_CACHE = {}


def _host_prep(x, w_pack, w_o):
    """Build per-core input maps (sharding + layout prep)."""
    x = np.asarray(x, dtype=np.float32)
    w_pack = np.asarray(w_pack, dtype=np.float32)
    w_o = np.asarray(w_o, dtype=np.float32)

    xT = np.ascontiguousarray(
        x.transpose(0, 2, 1).astype(ml_dtypes.bfloat16))   # [B, H, S] bf16

    inv_freq = 1.0 / (ROPE_BASE ** (np.arange(0, HD, 2, dtype=np.float32) / HD))
    t = np.arange(S, dtype=np.float32)
    freqs = np.outer(t, inv_freq)                            # [S, HD/2]
    emb = np.concatenate([freqs, freqs], axis=-1)            # [S, HD]
    cosT = np.ascontiguousarray(
        np.cos(emb).T.astype(ml_dtypes.bfloat16))            # [HD, S]
    sinT = np.sin(emb).T.astype(np.float32)
    sinTm = np.concatenate([-sinT[:HD // 2], sinT[HD // 2:]], axis=0)
    sinTm = np.ascontiguousarray(sinTm.astype(ml_dtypes.bfloat16))

    kk = np.arange(128)[:, None]
    qq = np.arange(QC)[None, :]
    masks = np.stack([
        np.where(kk + 128 * dj <= qq, 0.0, NEG).astype(np.float32)
        for dj in range(4)
    ]).astype(ml_dtypes.bfloat16)                             # [4, 128, QC]

    scale = float(HD) ** -0.5
    in_maps = []
    for c in range(NCORES):
        r0 = c * DPC
        wq = w_pack[r0:r0 + DPC, :] * scale                   # [512, H]
        wk = w_pack[H + r0:H + r0 + DPC, :]
        wv = w_pack[2 * H + r0:2 * H + r0 + DPC, :]
        # w slabs [12, 128 (h-part), NHT, 128 (cols)]: slab qkv*4+dt holds
        # rows dt*128:(dt+1)*128 of wq/wk/wv, indexed [h%128, h//128, col]
        wqk_slab = np.empty((12, 128, NHT, 128), dtype=ml_dtypes.bfloat16)
        for qk, wm in ((0, wq), (1, wk), (2, wv)):
            for dtt in range(HPC):
                blk = wm[dtt * 128:(dtt + 1) * 128, :]        # [128c, H]
                wqk_slab[qk * HPC + dtt] = np.ascontiguousarray(
                    blk.T.reshape(NHT, 128, 128).transpose(1, 0, 2)
                ).astype(ml_dtypes.bfloat16)
        woT = np.ascontiguousarray(
            w_o[:, r0:r0 + DPC].T.astype(ml_dtypes.bfloat16))  # [512, H]
        in_maps.append({
            "xT": xT, "wqk": wqk_slab, "woT": woT,
            "cosT": cosT, "sinTm": sinTm, "masks": masks,
        })
    return in_maps


def kernel(x, w_pack, w_o, _trace=False, _trace_kwargs=None):
    if "nc" not in _CACHE:
        _CACHE["nc"] = _build()
    nc = _CACHE["nc"]

    in_maps = _host_prep(x, w_pack, w_o)
    res = run_bass_kernel_spmd(nc, in_maps, list(range(NCORES)),
                               trace=_trace, **(_trace_kwargs or {}))
    acc = res.results[0]["out"].astype(np.float32)
    for c in range(1, NCORES):
        acc = acc + res.results[c]["out"].astype(np.float32)
    if _trace:
        kernel.last_results = res
    return acc


# revision 29
# speedup vs baseline: 1.1339x; 1.1339x over previous
"""Baichuan attention (B=2, S=2048, H=4096, 32 heads x 128) on 8 TRN2 NeuronCores.

Sharding: tensor-parallel over heads (4 heads per core), as in the original
model: W_pack column-sharded (per-head rows), o_proj row-sharded with the
partial-sum reduction done on the host during unshard ("all-reduce" of the
row-parallel output).

Fused single-region pipeline per core (all matmuls bf16), with attention
chunks interleaved into the dense QKV / o_proj matmul streams at emission
time so the PE queue always has independent work behind exp-latency stalls:

  qkv(b0) | attn(b0) x qkv(b1) | attn(b1) x oproj(b0) | oproj(b1)

  1. QKV projection from x^T: per (b, s-block): q/k passes as (qk, dt, sh)
     sub-passes - one [128,512] psum bank per sub-pass, stationary =
     [128,128] w-slab slice, LDW fully hidden at the 216ns N=512 cadence.
     RoPE on the fly (partition-swap via SBUF-SBUF DMA on the gpsimd
     queue).  qT/kT/v written bf16 to DRAM scratch.
  2. Attention per (b, h, qc), kt software-pipelined: scores matmul for
     kt+1 issued ahead of ctx/den matmuls for kt.  Denominator via all-ones
     stationary matmul (broadcast across partitions),
     reciprocal_approx_fast + normalize on VectorE.
  3. o_proj: ctx-stationary, drains alternate Scalar/Vector, bf16
     out-writes on the gpsimd DMA queue.
Host: shards/transposes inputs, sums the 8 row-parallel bf16 partials.
"""
import os
import sys

for _p in ("/opt/trn_rl_repo", "/root/.axon_site/_ro/trn_rl_repo"):
    if os.path.isdir(_p) and _p not in sys.path:
        sys.path.insert(0, _p)

from contextlib import ExitStack

import ml_dtypes
import numpy as np

import concourse.bass as bass
import concourse.tile as tile
from concourse import bacc, bass_isa, mybir
from concourse.bass_utils import run_bass_kernel_spmd

F32 = mybir.dt.float32
F32R = mybir.dt.float32r
BF16 = mybir.dt.bfloat16

B, S, H = 2, 2048, 4096
NH, HD = 32, 128
NCORES = 8
HPC = NH // NCORES          # heads per core = 4
DPC = HPC * HD              # dims per core = 512
ROPE_BASE = 10000.0
NEG = -1.0e30

SBLK = 1024                 # projection s-block
NSB = S // SBLK             # 2 s-blocks per batch
QC = 512                    # attention q-chunk
NQC = S // QC               # 4 q-chunks
NHT = H // 128              # 32 h-tiles (contraction tiles)
NKT = S // 128              # 16 k-tiles per sequence


def _build():
    nc = bacc.Bacc("TRN2", target_bir_lowering=False, debug=False,
                   num_devices=NCORES)

    xT = nc.dram_tensor("xT", [B, H, S], BF16, kind="ExternalInput").ap()
    # w slabs: [qkv(3) * dt(4)] slabs, each [128 (h-part), NHT, 128 (cols)]
    wqk = nc.dram_tensor("wqk", [12, 128, NHT, 128], BF16,
                         kind="ExternalInput").ap()
    woT = nc.dram_tensor("woT", [DPC, H], BF16, kind="ExternalInput").ap()
    cosT = nc.dram_tensor("cosT", [HD, S], BF16, kind="ExternalInput").ap()
    sinTm = nc.dram_tensor("sinTm", [HD, S], BF16, kind="ExternalInput").ap()
    masks = nc.dram_tensor("masks", [4, 128, QC], BF16,
                           kind="ExternalInput").ap()

    qkT_s = nc.dram_tensor("qkT_s", [B, 2 * DPC, S], BF16).ap()
    v_sT = nc.dram_tensor("v_sT", [B, DPC, S], BF16).ap()

    out = nc.dram_tensor("out", [B, S, H], BF16, kind="ExternalOutput").ap()

    with tile.TileContext(nc) as tc, ExitStack() as ctx:
        singles = ctx.enter_context(tc.tile_pool(name="singles", bufs=1))
        xpool = ctx.enter_context(tc.tile_pool(name="xslab", bufs=NHT))
        wpool = ctx.enter_context(tc.tile_pool(name="wslab", bufs=2))
        rpool = ctx.enter_context(tc.tile_pool(name="rope", bufs=2))
        opool = ctx.enter_context(tc.tile_pool(name="pj_out", bufs=3))
        qkvpool = ctx.enter_context(tc.tile_pool(name="at_qkv", bufs=1))
        ctxpool = ctx.enter_context(tc.tile_pool(name="at_ctx", bufs=2))
        prpool = ctx.enter_context(tc.tile_pool(name="at_pr", bufs=3))
        smpool = ctx.enter_context(tc.tile_pool(name="at_sm", bufs=2))
        wopool = ctx.enter_context(tc.tile_pool(name="at_wo", bufs=2))
        oopool = ctx.enter_context(tc.tile_pool(name="at_oo", bufs=4))
        # PSUM: 2 (qkv/v/oproj shared ring) + 2 scores + 2 ctx + 2 den = 8
        pp = ctx.enter_context(tc.tile_pool(name="ps_mm", bufs=2,
                                            space="PSUM"))
        ps_s = ctx.enter_context(tc.tile_pool(name="ps_s", bufs=3,
                                              space="PSUM"))
        ps_c = ctx.enter_context(tc.tile_pool(name="ps_c", bufs=2,
                                              space="PSUM"))
        ps_d = ctx.enter_context(tc.tile_pool(name="ps_d", bufs=1,
                                              space="PSUM"))

        cos_sb = singles.tile([HD, S], BF16)
        sin_sb = singles.tile([HD, S], BF16)
        nc.sync.dma_start(out=cos_sb[:], in_=cosT[:])
        nc.sync.dma_start(out=sin_sb[:], in_=sinTm[:])
        mask_sb = singles.tile([128, 4, QC], BF16)
        for dj in range(4):
            nc.sync.dma_start(out=mask_sb[:, dj, :], in_=masks[dj])
        ones_sb = singles.tile([128, 128], BF16)
        nc.vector.memset(ones_sb[:], 1.0)

        def drain_qk(ps, b, row0, s0):
            """rope(psum tile [128, 512]) -> qkT_s[b, row0:+128, s0:+512]"""
            sl = ps[:]  # [128, 512]
            cslice = cos_sb[:, s0:s0 + 512]
            mslice = sin_sb[:, s0:s0 + 512]
            t1 = rpool.tile([128, 512], F32, tag="t1")
            nc.vector.tensor_mul(t1[:], sl, cslice)
            qsb = rpool.tile([128, 512], BF16, tag="qsb")
            nc.scalar.copy(qsb[:], sl)
            qsw = rpool.tile([128, 512], BF16, tag="qsw")
            nc.gpsimd.dma_start(out=qsw[0:64, :], in_=qsb[64:128, :])
            nc.gpsimd.dma_start(out=qsw[64:128, :], in_=qsb[0:64, :])
            nc.vector.tensor_mul(qsw[:], qsw[:], mslice)
            qo = opool.tile([128, 512], BF16, tag="qo")
            nc.vector.tensor_add(qo[:], t1[:], qsw[:])
            nc.gpsimd.dma_start(out=qkT_s[b, row0:row0 + 128, s0:s0 + 512],
                                in_=qo[:])

        def qkv_units(b, sb):
            """Emission units (closures) for one (b, s-block)."""
            s0 = sb * SBLK
            xsl = []

            def load_x():
                for h in range(NHT):
                    xs = xpool.tile([128, SBLK], BF16, tag="xs", name="xs")
                    nc.sync.dma_start(
                        out=xs[:],
                        in_=xT[b, h * 128:(h + 1) * 128, s0:s0 + SBLK])
                    xsl.append(xs)

            units = [load_x]

            # q/k/v passes: one psum bank per (pass, dt, sh) sub-pass
            for qkv in range(3):
                for dt in range(HPC):
                    def mk(qkv=qkv, dt=dt):
                        def run():
                            w = wpool.tile([128, NHT, 128], BF16, tag="w")
                            nc.sync.dma_start(out=w[:],
                                              in_=wqk[qkv * HPC + dt])
                            for sh in range(2):
                                ps = pp.tile([128, 512], F32, tag="mm",
                                             name="pp")
                                for h in range(NHT):
                                    nc.tensor.matmul(
                                        ps[:],
                                        w[:, h, :],
                                        xsl[h][:, sh * 512:(sh + 1) * 512],
                                        start=(h == 0), stop=(h == NHT - 1))
                                if qkv < 2:
                                    drain_qk(ps, b, qkv * DPC + dt * 128,
                                             s0 + sh * 512)
                                else:
                                    vo = opool.tile([128, 512], BF16,
                                                    tag="vo")
                                    nc.scalar.copy(vo[:], ps[:])
                                    nc.gpsimd.dma_start(
                                        out=v_sT[b,
                                                 dt * 128:(dt + 1) * 128,
                                                 s0 + sh * 512:
                                                 s0 + (sh + 1) * 512],
                                        in_=vo[:])
                        return run
                    units.append(mk())
            return units

        def attn_units(b, qT_all, kT_all, v_sb, ctx_sb, den_pe=False):
            """Emission units: one per (qc, h) chunk, qc-major order."""
            units = []
            for qc in range(NQC):
                for h in range(HPC):
                    def mk(h=h, qc=qc):
                        def run():
                            q0 = qc * QC
                            nkt = 4 * qc + 4
                            pc = ps_c.tile([128, QC], F32, tag="pc",
                                           name="pc")
                            den = smpool.tile([128, QC], F32, tag="den")
                            if den_pe:
                                pden = ps_d.tile([128, QC], F32, tag="pd",
                                                 name="pd")

                            def w0_of(kt):
                                # first unmasked q column of this k-tile
                                return max(0, (kt - 4 * qc) * 128)

                            def acc_prev(prev):
                                # ctx matmul + den accumulation for slot kt
                                pkt, ppr = prev
                                w0 = w0_of(pkt)
                                nc.tensor.matmul(
                                    pc[:, w0:QC],
                                    v_sb[:, pkt, h * HD:(h + 1) * HD],
                                    ppr[:, w0:QC], start=(pkt == 0),
                                    stop=(pkt == nkt - 1))
                                if den_pe:
                                    nc.tensor.matmul(
                                        pden[:, w0:QC], ones_sb[:],
                                        ppr[:, w0:QC], start=(pkt == 0),
                                        stop=(pkt == nkt - 1))
                                elif pkt == 0:
                                    nc.vector.tensor_copy(den[:], ppr[:])
                                else:
                                    nc.vector.tensor_add(
                                        den[:, w0:QC], den[:, w0:QC],
                                        ppr[:, w0:QC])

                            prev = None
                            for kt in range(nkt):
                                w0 = w0_of(kt)
                                pss = ps_s.tile([128, QC], F32, tag="pss",
                                                name="pss")
                                nc.tensor.matmul(
                                    pss[:, w0:QC],
                                    kT_all[:, h, kt * 128:(kt + 1) * 128],
                                    qT_all[:, h, q0 + w0:q0 + QC],
                                    start=True, stop=True)
                                if kt >= 4 * qc:
                                    nc.vector.tensor_add(
                                        pss[:, w0:QC], pss[:, w0:QC],
                                        mask_sb[:, kt - 4 * qc, w0:QC])
                                pr = prpool.tile([128, QC], BF16, tag="pr",
                                                 name="pr")
                                nc.scalar.activation(
                                    out=pr[:, w0:QC], in_=pss[:, w0:QC],
                                    func=mybir.ActivationFunctionType.Exp)
                                if prev is not None:
                                    acc_prev(prev)
                                prev = (kt, pr)
                            acc_prev(prev)
                            if not den_pe:
                                den16 = smpool.tile([128, QC], BF16,
                                                    tag="den16")
                                nc.scalar.copy(den16[:], den[:])
                                pden = ps_d.tile([128, QC], F32, tag="pd",
                                                 name="pd")
                                nc.tensor.matmul(pden[:], ones_sb[:],
                                                 den16[:],
                                                 start=True, stop=True)
                            nc.vector.reciprocal_approx_fast(out=den[:],
                                                             in_=pden[:])
                            nc.vector.tensor_mul(
                                ctx_sb[:, h, q0:q0 + QC], pc[:], den[:])
                        return run
                    units.append(mk())
            return units

        def oproj_units(b, ctx_sb, st_lo=0, st_hi=S // 128):
            units = []
            for oc in range(H // 512):
                def mk(oc=oc):
                    def run():
                        wos = wopool.tile([128, HPC, 512], BF16, tag="wos")
                        nc.sync.dma_start(
                            out=wos[:],
                            in_=woT[:, oc * 512:(oc + 1) * 512].rearrange(
                                "(h p) o -> p h o", p=128))
                        for st in range(st_lo, st_hi):
                            po = pp.tile([128, 512], F32, tag="mm",
                                         name="po")
                            for h in range(HPC):
                                nc.tensor.matmul(
                                    po[:],
                                    ctx_sb[:, h, st * 128:(st + 1) * 128],
                                    wos[:, h, :],
                                    start=(h == 0), stop=(h == HPC - 1))
                            ot = oopool.tile([128, 512], BF16, tag="ot")
                            if st % 2 == 0:
                                nc.scalar.copy(ot[:], po[:])
                            else:
                                nc.vector.tensor_copy(ot[:], po[:])
                            nc.gpsimd.dma_start(
                                out=out[b, st * 128:(st + 1) * 128,
                                        oc * 512:(oc + 1) * 512],
                                in_=ot[:])
                    return run
                units.append(mk())
            return units

        def alloc_attn_inputs():
            qT_all = qkvpool.tile([128, HPC, S], BF16, tag="qT")
            kT_all = qkvpool.tile([128, HPC, S], BF16, tag="kT")
            v_sb = qkvpool.tile([128, NKT, DPC], BF16, tag="v")
            return qT_all, kT_all, v_sb

        def load_attn_inputs(b, tiles, qcs):
            # per-qc-slice loads, emitted only after the producing s-block
            qT_all, kT_all, v_sb = tiles
            for qc in qcs:
                sl = slice(qc * QC, (qc + 1) * QC)
                nc.sync.dma_start(
                    out=qT_all[:, :, sl],
                    in_=qkT_s[b, 0:DPC, sl].rearrange(
                        "(h p) s -> p h s", p=128))
                nc.sync.dma_start(
                    out=kT_all[:, :, sl],
                    in_=qkT_s[b, DPC:2 * DPC, sl].rearrange(
                        "(h p) s -> p h s", p=128))
                for kt in range(4 * qc, 4 * qc + 4):
                    nc.sync.dma_start_transpose(
                        out=v_sb[:, kt, :],
                        in_=v_sT[b][:, kt * 128:(kt + 1) * 128])

        def interleave(primary, secondary):
            """Emit primary units with secondary units spread between them."""
            if not secondary:
                for u in primary:
                    u()
                return
            ratio = len(primary) / len(secondary)
            si = 0
            for i, u in enumerate(primary):
                u()
                while si < len(secondary) and si + 1 <= (i + 1) / ratio:
                    secondary[si]()
                    si += 1
            while si < len(secondary):
                secondary[si]()
                si += 1

        # ---- fused schedule ----
        # qkv(0,0) | qkv(0,1) x attn0[qc01] | qkv(1) x attn0[qc23] |
        # oproj(0) x attn1 | oproj(1)
        for u in qkv_units(0, 0):
            u()
        at0 = alloc_attn_inputs()
        ctx0 = ctxpool.tile([128, HPC, S], BF16, tag="ctx")
        a0 = attn_units(0, *at0, ctx0)       # (qc, h) order, 4 per qc
        u01 = qkv_units(0, 1)
        u01[0]()                             # x loads for block (0,1)
        load_attn_inputs(0, at0, (0, 1))
        interleave(u01[1:], a0[:8])
        qkv1 = [u for sb in range(NSB) for u in qkv_units(1, sb)]
        qkv1[0]()                            # x loads for block (1,0)
        load_attn_inputs(0, at0, (2, 3))
        # B: qkv(1) filled with attn0 qc2/3 chunks and the first st-half of
        # oproj(0) (its ctx rows are complete once the qc0/1 chunks ran)
        op0a = oproj_units(0, ctx0, 0, 8)
        op0b = oproj_units(0, ctx0, 8, 16)
        secB = []
        for i in range(8):
            secB.append(a0[8 + i])
            secB.append(op0a[i])
        interleave(qkv1[1:], secB)
        at1 = alloc_attn_inputs()
        ctx1 = ctxpool.tile([128, HPC, S], BF16, tag="ctx")
        op0b[0]()
        load_attn_inputs(1, at1, (0, 1, 2, 3))
        interleave(op0b[1:], attn_units(1, *at1, ctx1))
        for u in oproj_units(1, ctx1):
            u()

    nc.compile()
    return nc


_CACHE = {}


def _host_prep(x, w_pack, w_o):
    """Build per-core input maps (sharding + layout prep)."""
    x = np.asarray(x, dtype=np.float32)
    w_pack = np.asarray(w_pack, dtype=np.float32)
    w_o = np.asarray(w_o, dtype=np.float32)

    xT = np.ascontiguousarray(
        x.transpose(0, 2, 1).astype(ml_dtypes.bfloat16))   # [B, H, S] bf16

    inv_freq = 1.0 / (ROPE_BASE ** (np.arange(0, HD, 2, dtype=np.float32) / HD))
    t = np.arange(S, dtype=np.float32)
    freqs = np.outer(t, inv_freq)                            # [S, HD/2]
    emb = np.concatenate([freqs, freqs], axis=-1)            # [S, HD]
    cosT = np.ascontiguousarray(
        np.cos(emb).T.astype(ml_dtypes.bfloat16))            # [HD, S]
    sinT = np.sin(emb).T.astype(np.float32)
    sinTm = np.concatenate([-sinT[:HD // 2], sinT[HD // 2:]], axis=0)
    sinTm = np.ascontiguousarray(sinTm.astype(ml_dtypes.bfloat16))

    kk = np.arange(128)[:, None]
    qq = np.arange(QC)[None, :]
    masks = np.stack([
        np.where(kk + 128 * dj <= qq, 0.0, NEG).astype(np.float32)
        for dj in range(4)
    ]).astype(ml_dtypes.bfloat16)                             # [4, 128, QC]

    scale = float(HD) ** -0.5
    in_maps = []
    for c in range(NCORES):
        r0 = c * DPC
        wq = w_pack[r0:r0 + DPC, :] * scale                   # [512, H]
        wk = w_pack[H + r0:H + r0 + DPC, :]
        wv = w_pack[2 * H + r0:2 * H + r0 + DPC, :]
        # w slabs [12, 128 (h-part), NHT, 128 (cols)]: slab qkv*4+dt holds
        # rows dt*128:(dt+1)*128 of wq/wk/wv, indexed [h%128, h//128, col]
        wqk_slab = np.empty((12, 128, NHT, 128), dtype=ml_dtypes.bfloat16)
        for qk, wm in ((0, wq), (1, wk), (2, wv)):
            for dtt in range(HPC):
                blk = wm[dtt * 128:(dtt + 1) * 128, :]        # [128c, H]
                wqk_slab[qk * HPC + dtt] = np.ascontiguousarray(
                    blk.T.reshape(NHT, 128, 128).transpose(1, 0, 2)
                ).astype(ml_dtypes.bfloat16)
        woT = np.ascontiguousarray(
            w_o[:, r0:r0 + DPC].T.astype(ml_dtypes.bfloat16))  # [512, H]
        in_maps.append({
            "xT": xT, "wqk": wqk_slab, "woT": woT,
            "cosT": cosT, "sinTm": sinTm, "masks": masks,
        })
    return in_maps


def kernel(x, w_pack, w_o, _trace=False, _trace_kwargs=None):
    if "nc" not in _CACHE:
        _CACHE["nc"] = _build()
    nc = _CACHE["nc"]

    in_maps = _host_prep(x, w_pack, w_o)
    res = run_bass_kernel_spmd(nc, in_maps, list(range(NCORES)),
                               trace=_trace, **(_trace_kwargs or {}))
    acc = res.results[0]["out"].astype(np.float32)
    for c in range(1, NCORES):
        acc = acc + res.results[c]["out"].astype(np.float32)
    if _trace:
        kernel.last_results = res
    return acc
